# revision 23
# baseline (speedup 1.0000x reference)
"""Trainium2 Bass kernel for an 8-layer GPT-2-style transformer (NanoShakes).

Sharding: 8-way sequence-data-parallel. The 4x1024 tokens are split into 8
chunks of 512 (4 batch rows x 2 halves). Core 2r holds the first half of row
r, core 2r+1 the second half. Attention for second-half queries needs the
first half's K/V, exchanged with a pairwise AllGather each layer; first-half
cores mask out the (unneeded) remote block via an additive -1e9 exp bias, so
one uniform SPMD program runs on all cores.

On-device layout: activations are kept feature-major ("transposed", [E, T])
so every dense matmul uses the natural [E_in, E_out] weight as the stationary
operand and never needs an on-chip transpose. LayerNorm gains are folded into
the following weight matrix on the host; the LN mean subtraction and all
biases are applied through augmented contraction rows (extra k-rows of the
matmul). Matmul compute is bf16 with f32 accumulation; the residual stream
stays f32.
"""

import sys

sys.path.insert(0, "/opt/trn_rl_repo")

import contextlib

import numpy as np
import ml_dtypes

import concourse.bass as bass
import concourse.bacc as bacc
import concourse.tile as tile
from concourse import mybir
from concourse.bass_utils import run_bass_kernel_spmd

BF16 = ml_dtypes.bfloat16
F32 = mybir.dt.float32
BF = mybir.dt.bfloat16

V, T, E, L, H = 32000, 1024, 1024, 8, 16
DH = E // H
EPS = 1e-5
B = 4
NCORES = 8
TC = 512          # tokens per core
KT = 8            # k-tiles over E
NEG = -1.0e9

_CACHE = {}


# ----------------------------------------------------------------------------
# device program
# ----------------------------------------------------------------------------

def build_nc(n_layers=L, include_head=True, debug_tap=None):
    nc = bacc.Bacc("TRN2", target_bir_lowering=False, debug=False,
                   num_devices=NCORES)

    LD = max(n_layers, 1)
    x0T_d = nc.dram_tensor("x0T", [E, TC], F32, kind="ExternalInput")
    wqk_d = nc.dram_tensor("wqk", [LD, E + 2, 2 * E], BF, kind="ExternalInput")
    wv_d = nc.dram_tensor("wv", [LD, E + 2, E], BF, kind="ExternalInput")
    wpr_d = nc.dram_tensor("wpr", [LD, E + 1, E], BF, kind="ExternalInput")
    wf1_d = nc.dram_tensor("wf1", [LD, E + 2, 4 * E], BF, kind="ExternalInput")
    wf2_d = nc.dram_tensor("wf2", [LD, 4 * E + 1, E], BF, kind="ExternalInput")
    wh_d = (nc.dram_tensor("wh", [E + 2, V], BF, kind="ExternalInput")
            if include_head else None)
    mask_d = nc.dram_tensor("mask_tri", [128, 128], F32, kind="ExternalInput")
    rb_d = nc.dram_tensor("rem_bias", [128, 1], F32, kind="ExternalInput")
    if include_head:
        out_d = nc.dram_tensor("out", [TC, V], F32, kind="ExternalOutput")
    elif debug_tap is not None:
        out_d = nc.dram_tensor("out", [4096, TC], F32, kind="ExternalOutput")
    else:
        out_d = nc.dram_tensor("out", [E, TC], F32, kind="ExternalOutput")

    with tile.TileContext(nc) as tc:
        _body(nc, tc, n_layers, include_head,
              x0T_d, wqk_d, wv_d, wpr_d, wf1_d, wf2_d, wh_d, mask_d, rb_d,
              out_d, debug_tap)
    nc.compile()
    return nc


def _body(nc, tc, n_layers, include_head,
          x0T_d, wqk_d, wv_d, wpr_d, wf1_d, wf2_d, wh_d, mask_d, rb_d, out_d,
          debug_tap=None):
    AG = mybir.AluOpType
    AF = mybir.ActivationFunctionType
    ts = bass.ts

    ctx = contextlib.ExitStack()
    with ctx:
        persist = ctx.enter_context(tc.tile_pool(name="persist", bufs=1))
        act = ctx.enter_context(tc.tile_pool(name="act", bufs=1))
        wstr = ctx.enter_context(tc.tile_pool(name="wstr", bufs=2))
        wf2p = ctx.enter_context(tc.tile_pool(name="wf2p", bufs=4))
        waug = ctx.enter_context(tc.tile_pool(name="waug", bufs=2))
        ppool = ctx.enter_context(tc.tile_pool(name="ppool", bufs=3))
        small = ctx.enter_context(tc.tile_pool(name="small", bufs=2))
        rbp = ctx.enter_context(tc.tile_pool(name="rbp", bufs=2))
        lop = ctx.enter_context(tc.tile_pool(name="lop", bufs=3))
        dram = ctx.enter_context(tc.tile_pool(name="dram", bufs=2,
                                              space="DRAM"))
        psp = ctx.enter_context(tc.tile_pool(name="psp", bufs=8,
                                             space="PSUM"))

        def ptile():
            return psp.tile([128, TC], F32, tag="ps", name="ps")

        # ---- persistent tiles -------------------------------------------
        xT = persist.tile([128, KT, TC], F32)         # residual stream
        ones_col_f = persist.tile([128, 1], F32)      # lhsT for col-sums
        ones_col_b = persist.tile([128, 1], BF)
        ones_row_f = persist.tile([1, 128], F32)      # lhsT for broadcasts
        ones_row_b = persist.tile([1, TC], BF)        # rhs aug row (biases)
        mask_tri = persist.tile([128, 128], F32)
        rem_bias = persist.tile([128, 1], F32)
        zero_bias = persist.tile([128, 1], F32)
        eps_tile = persist.tile([1, 1], F32)
        nc.vector.memset(eps_tile[:], EPS)

        nc.vector.memset(ones_col_f[:], 1.0)
        nc.vector.memset(ones_col_b[:], 1.0)
        nc.vector.memset(ones_row_f[:], 1.0)
        nc.vector.memset(ones_row_b[:], 1.0)
        nc.vector.memset(zero_bias[:], 0.0)
        nc.sync.dma_start(mask_tri[:], mask_d[:])
        nc.sync.dma_start(rem_bias[:], rb_d[:])

        # load embedding (transposed):  [E, TC] -> [128, KT, TC]
        nc.sync.dma_start(
            xT[:], x0T_d.ap().rearrange("(kt p) t -> p kt t", p=128))

        # ------------------------------------------------------------------
        def layernorm():
            """LN over the partition (feature) axis of xT.

            Returns (xa, augx): xa = (x * rstd) in bf16, and augx [2, TC]
            rhs-rows [c; ones] with c = -mu * rstd; the following matmul's
            host-prepared weights carry the matching column-sum and bias
            rows that complete the normalization.
            """
            s1 = ptile()
            s2 = ptile()
            for k in range(KT):
                sq = ppool.tile([128, TC], BF, tag="sq")
                nc.scalar.activation(sq[:], xT[:, k, :], AF.Square)
                nc.tensor.matmul(s1[0:1, :], ones_col_f[:], xT[:, k, :],
                                 start=(k == 0), stop=(k == KT - 1))
                nc.tensor.matmul(s2[0:1, :], ones_col_b[:], sq[:],
                                 start=(k == 0), stop=(k == KT - 1))
            mu = small.tile([1, TC], F32, tag="mu")
            var = small.tile([1, TC], F32, tag="var")
            a = small.tile([1, TC], F32, tag="a")
            tmp = small.tile([1, TC], F32, tag="tmp")
            nc.vector.tensor_scalar_mul(mu[:], s1[0:1, :], 1.0 / E)
            nc.vector.tensor_scalar_mul(var[:], s2[0:1, :], 1.0 / E)
            nc.vector.tensor_tensor(tmp[:], mu[:], mu[:], AG.mult)
            nc.vector.tensor_tensor(var[:], var[:], tmp[:], AG.subtract)
            # rstd = exp(-0.5 * ln(var + eps))  (avoids sqrt table swap)
            nc.scalar.activation(tmp[:], var[:], AF.Ln, bias=eps_tile[:])
            nc.scalar.activation(a[:], tmp[:], AF.Exp, scale=-0.5)
            augx = small.tile([2, TC], BF, tag="augx")
            nc.vector.tensor_tensor(tmp[:], mu[:], a[:], AG.mult)
            nc.vector.tensor_scalar_mul(augx[0:1, :], tmp[:], -1.0)
            nc.sync.dma_start(augx[1:2, :], ones_row_b[0:1, :])
            # broadcast a over partitions (k=1 matmul), then apply
            ab = ptile()
            nc.tensor.matmul(ab[:], ones_row_f[:], a[:], start=True, stop=True)
            xa = act.tile([128, KT, TC], BF, tag="xa")
            for k in range(KT):
                nc.vector.tensor_tensor(xa[:, k, :], xT[:, k, :], ab[:],
                                        AG.mult)
            return xa, augx

        # ------------------------------------------------------------------
        def dense_wT(xa, augx, w_dram, waug_dram, out_cb, n_cols, aug_k):
            """out_T = W'.T @ xhat (+ aug rows), W' streamed in 1024-col
            chunks. out_cb(m_global, psum_tile) consumes each 128-row
            output tile."""
            n_chunks = n_cols // 1024
            for ch in range(n_chunks):
                cs = slice(1024 * ch, 1024 * (ch + 1))
                w_t = wstr.tile([128, KT, 1024], BF, tag="wstream")
                nc.sync.dma_start(
                    w_t[:],
                    w_dram[0:E, cs].rearrange("(kt p) n -> p kt n", p=128))
                w_a = waug.tile([aug_k, 1024], BF, tag="waug")
                nc.sync.dma_start(w_a[:], waug_dram[:, cs])
                for mm in range(8):
                    ps = ptile()
                    for k in range(KT):
                        nc.tensor.matmul(ps[:], w_t[:, k, ts(mm, 128)],
                                         xa[:, k, :],
                                         start=(k == 0), stop=False)
                    nc.tensor.matmul(
                        ps[:], w_a[:, ts(mm, 128)],
                        augx[:] if aug_k == 2 else ones_row_b[0:1, :],
                        start=False, stop=True)
                    out_cb(8 * ch + mm, ps)

        def dbg_rows(r0, nrows):
            return out_d[r0:r0 + nrows, :]

        # ------------------------------------------------------------------
        for l in range(n_layers):
            # ================= LN1 + QKV =================================
            xa, augx = layernorm()
            if debug_tap == "xa" and l == 0:
                nc.gpsimd.dma_start(
                    dbg_rows(0, E).rearrange("(kt p) t -> p kt t", p=128),
                    xa[:])
                nc.gpsimd.dma_start(dbg_rows(3072, 2), augx[:])

            qk = act.tile([128, 16, TC], BF, tag="qk")

            def qk_out(m, ps):
                nc.vector.tensor_copy(qk[:, m, :], ps[:])

            dense_wT(xa, augx, wqk_d[l], wqk_d[l, E:E + 2, :], qk_out,
                     2 * E, 2)
            if debug_tap == "qk" and l == 0:
                nc.gpsimd.dma_start(
                    dbg_rows(0, 2 * E).rearrange("(kt p) t -> p kt t", p=128),
                    qk[:])

            # V in natural layout [tokens, E] with an extra ones column per
            # head (so the PV matmul also emits the softmax denominator).
            vloc = act.tile([128, 4, H, DH + 1], BF, tag="vloc")
            for tt in range(4):
                nc.vector.memset(vloc[:, tt, :, DH:DH + 1], 1.0)
            for nv in range(2):
                wv_t = wstr.tile([128, KT, 512], BF, tag="wstream")
                nc.sync.dma_start(
                    wv_t[:],
                    wv_d[l, 0:E, ts(nv, 512)].rearrange(
                        "(kt p) n -> p kt n", p=128))
                wv_aug = waug.tile([2, 512], BF, tag="waug")
                nc.sync.dma_start(wv_aug[:], wv_d[l, E:E + 2, ts(nv, 512)])
                for tt in range(4):
                    ps = ptile()
                    for k in range(KT):
                        nc.tensor.matmul(ps[:], xa[:, k, ts(tt, 128)],
                                         wv_t[:, k, :],
                                         start=(k == 0), stop=False)
                    nc.tensor.matmul(ps[:], augx[:, ts(tt, 128)], wv_aug[:],
                                     start=False, stop=True)
                    nc.vector.tensor_copy(
                        vloc[:, tt, ts(nv, 8), 0:DH],
                        ps[:].rearrange("p (h d) -> p h d", d=DH))

            if debug_tap == "v" and l == 0:
                for i in range(4):
                    nc.gpsimd.dma_start(
                        dbg_rows(256 * i, 256).rearrange(
                            "(p two) c -> p (two c)", two=2),
                        vloc[:, i, :, 0:DH])

            # ================= K/V exchange ==============================
            kv_send = dram.tile([2048, TC], BF, tag="kv_send")
            for j in range(8):
                nc.sync.dma_start(kv_send[128 * j:128 * (j + 1), :],
                                  qk[:, 8 + j, :])
            for i in range(4):
                nc.sync.dma_start(
                    kv_send[1024 + 256 * i:1024 + 256 * (i + 1), :]
                    .rearrange("(p two) c -> p (two c)", two=2),
                    vloc[:, i, :, 0:DH])
            kv_all = dram.tile([2, 2048, TC], BF, tag="kv_all")
            nc.gpsimd.collective_compute(
                "AllGather", AG.bypass,
                replica_groups=[[0, 1], [2, 3], [4, 5], [6, 7]],
                ins=[kv_send[:].opt()],
                outs=[kv_all[:].opt()],
            )
            krem = act.tile([128, 8, TC], BF, tag="krem")
            for j in range(8):
                nc.sync.dma_start(krem[:, j, :],
                                  kv_all[0, 128 * j:128 * (j + 1), :])
            vrem = act.tile([128, 4, H, DH + 1], BF, tag="vrem")
            for i in range(4):
                nc.vector.memset(vrem[:, i, :, DH:DH + 1], 1.0)
                nc.sync.dma_start(
                    vrem[:, i, :, 0:DH],
                    kv_all[0, 1024 + 256 * i:1024 + 256 * (i + 1), :]
                    .rearrange("(p two) c -> p (two c)", two=2))

            # ================= attention =================================
            oT = act.tile([128, KT, TC], BF, tag="oT")
            for h in range(H):
                pb = (h % 2) * 64
                q_h = qk[pb:pb + 64, h // 2, :]
                o_aug = ptile()
                # remote k-tiles (keys 0..511 of the pair's first half)
                for i in range(4):
                    st = ptile()
                    nc.tensor.matmul(st[:], krem[pb:pb + 64, h // 2,
                                                 ts(i, 128)],
                                     q_h, start=True, stop=True)
                    p = ppool.tile([128, TC], BF, tag="p")
                    nc.scalar.activation(p[:], st[:], AF.Exp,
                                         bias=rem_bias[:])
                    nc.tensor.matmul(o_aug[0:65, :], vrem[:, i, h, :], p[:],
                                     start=(i == 0), stop=False)
                # local k-tiles (causal)
                for j in range(4):
                    n = TC - 128 * j
                    st = ptile()
                    nc.tensor.matmul(st[:, 0:n],
                                     qk[pb:pb + 64, 8 + h // 2, ts(j, 128)],
                                     q_h[:, 128 * j:TC],
                                     start=True, stop=True)
                    nc.vector.tensor_tensor(st[:, 0:128], st[:, 0:128],
                                            mask_tri[:], AG.add)
                    p = ppool.tile([128, TC], BF, tag="p")
                    nc.scalar.activation(p[:, 0:n], st[:, 0:n], AF.Exp,
                                         bias=zero_bias[:])
                    nc.tensor.matmul(o_aug[0:65, 128 * j:TC],
                                     vloc[:, j, h, :], p[:, 0:n],
                                     start=False, stop=(j == 3))
                # normalize: o / den  (den = row 64 of o_aug)
                den_sb = rbp.tile([1, TC], F32, tag="den_sb")
                nc.vector.tensor_copy(den_sb[:], o_aug[64:65, :])
                rden = rbp.tile([1, TC], F32, tag="rden")
                nc.vector.reciprocal_approx_fast(rden[:], den_sb[:])
                rbps = ptile()
                nc.tensor.matmul(rbps[0:64, :], ones_row_f[0:1, 0:64],
                                 rden[:], start=True, stop=True)
                rb = rbp.tile([64, TC], F32, tag="rb")
                nc.scalar.copy(rb[:], rbps[0:64, :])
                nc.vector.tensor_tensor(oT[pb:pb + 64, h // 2, :],
                                        o_aug[0:64, :], rb[:], AG.mult)
                if debug_tap == "att0" and l == 0 and h == 0:
                    dbg = persist.tile([128, 3, TC], F32, name="dbg")
                    nc.vector.tensor_copy(dbg[0:65, 0, :], o_aug[0:65, :])
                    nc.vector.tensor_copy(dbg[0:1, 1, :], rden[:])
                    nc.vector.tensor_copy(dbg[0:64, 2, :], rb[:])
                    nc.gpsimd.dma_start(
                        dbg_rows(0, 384).rearrange("(k p) t -> p k t", p=128),
                        dbg[:])

            if debug_tap == "o" and l == 0:
                nc.gpsimd.dma_start(
                    dbg_rows(0, E).rearrange("(kt p) t -> p kt t", p=128),
                    oT[:])

            # ================= proj + residual ===========================
            def proj_out(m, ps):
                nc.vector.tensor_tensor(xT[:, m, :], xT[:, m, :], ps[:],
                                        AG.add)

            dense_wT(oT, None, wpr_d[l], wpr_d[l, E:E + 1, :], proj_out,
                     E, 1)

            # ================= LN2 + FFN =================================
            xa2, augx2 = layernorm()
            hT0 = act.tile([128, 16, TC], BF, tag="hT0")
            hT1 = act.tile([128, 16, TC], BF, tag="hT1")

            def ff1_out(m, ps):
                dst = hT0 if m < 16 else hT1
                nc.scalar.activation(dst[:, m % 16, :], ps[:], AF.Relu)

            dense_wT(xa2, augx2, wf1_d[l], wf1_d[l, E:E + 2, :], ff1_out,
                     4 * E, 2)

            ffps = [ptile() for _ in range(KT)]
            for kg in range(32):
                wf2 = wf2p.tile([128, E], BF, tag="wf2")
                nc.sync.dma_start(wf2[:],
                                  wf2_d[l, 128 * kg:128 * (kg + 1), :])
                src = hT0 if kg < 16 else hT1
                for m in range(KT):
                    nc.tensor.matmul(ffps[m][:], wf2[:, ts(m, 128)],
                                     src[:, kg % 16, :],
                                     start=(kg == 0), stop=False)
            wf2_aug = waug.tile([1, E], BF, tag="waug")
            nc.sync.dma_start(wf2_aug[:], wf2_d[l, 4 * E:4 * E + 1, :])
            for m in range(KT):
                nc.tensor.matmul(ffps[m][:], wf2_aug[:, ts(m, 128)],
                                 ones_row_b[0:1, :], start=False, stop=True)
                nc.vector.tensor_tensor(xT[:, m, :], xT[:, m, :],
                                        ffps[m][:], AG.add)

        # ================= final LN + head ===============================
        if debug_tap is not None:
            return
        if not include_head:
            for k in range(KT):
                nc.sync.dma_start(
                    out_d.ap().rearrange("(kt p) t -> p kt t", p=128)[:, k, :],
                    xT[:, k, :])
            return

        xaf, augxf = layernorm()
        vchunks = [(i * 512, 512) for i in range(62)] + [(62 * 512, 256)]
        for ci, (v0, vn) in enumerate(vchunks):
            wh_t = wstr.tile([128, KT, 512], BF, tag="wstream")
            nc.sync.dma_start(
                wh_t[:, :, 0:vn],
                wh_d[0:E, v0:v0 + vn].rearrange("(kt p) n -> p kt n", p=128))
            wh_aug = waug.tile([2, 512], BF, tag="waug")
            nc.sync.dma_start(wh_aug[:, 0:vn], wh_d[E:E + 2, v0:v0 + vn])
            for tt in range(4):
                ps = ptile()
                for k in range(KT):
                    nc.tensor.matmul(ps[:, 0:vn], xaf[:, k, ts(tt, 128)],
                                     wh_t[:, k, 0:vn],
                                     start=(k == 0), stop=False)
                nc.tensor.matmul(ps[:, 0:vn], augxf[:, ts(tt, 128)],
                                 wh_aug[:, 0:vn], start=False, stop=True)
                lo = lop.tile([128, 512], F32, tag="lo")
                if tt % 2 == 0:
                    nc.vector.tensor_copy(lo[:, 0:vn], ps[:, 0:vn])
                else:
                    nc.scalar.copy(lo[:, 0:vn], ps[:, 0:vn])
                nc.sync.dma_start(out_d[ts(tt, 128), v0:v0 + vn],
                                  lo[:, 0:vn])


# ----------------------------------------------------------------------------
# host side
# ----------------------------------------------------------------------------

def _prep_weights(inputs):
    """Fold LN gains/biases into weights, build augmented rows, cast bf16."""
    f = lambda k: np.asarray(inputs[k], np.float32)
    qkv_w, proj_w, proj_b = f("qkv_w"), f("proj_w"), f("proj_b")
    ff1_w, ff1_b, ff2_w, ff2_b = f("ff1_w"), f("ff1_b"), f("ff2_w"), f("ff2_b")
    ln1_g, ln1_b = f("ln1_g"), f("ln1_b")
    ln2_g, ln2_b = f("ln2_g"), f("ln2_b")
    lnf_g, lnf_b = f("lnf_g"), f("lnf_b")
    head_w, head_b = f("head_w"), f("head_b")

    qscale = DH ** -0.5

    wqk = np.empty((L, E + 2, 2 * E), np.float32)
    wv = np.empty((L, E + 2, E), np.float32)
    wpr = np.empty((L, E + 1, E), np.float32)
    wf1 = np.empty((L, E + 2, 4 * E), np.float32)
    wf2 = np.empty((L, 4 * E + 1, E), np.float32)
    for l in range(L):
        wqk_s = qkv_w[l][:, :2 * E].copy()
        wqk_s[:, :E] *= qscale
        wqk_f = ln1_g[l][:, None] * wqk_s
        wqk[l, :E] = wqk_f
        wqk[l, E] = wqk_f.sum(0)
        wqk[l, E + 1] = ln1_b[l] @ wqk_s

        wv_s = qkv_w[l][:, 2 * E:]
        wv_f = ln1_g[l][:, None] * wv_s
        wv[l, :E] = wv_f
        wv[l, E] = wv_f.sum(0)
        wv[l, E + 1] = ln1_b[l] @ wv_s

        wpr[l, :E] = proj_w[l]
        wpr[l, E] = proj_b[l]

        wf1_f = ln2_g[l][:, None] * ff1_w[l]
        wf1[l, :E] = wf1_f
        wf1[l, E] = wf1_f.sum(0)
        wf1[l, E + 1] = ln2_b[l] @ ff1_w[l] + ff1_b[l]

        wf2[l, :4 * E] = ff2_w[l]
        wf2[l, 4 * E] = ff2_b[l]

    wh = np.empty((E + 2, V), np.float32)
    wh_f = lnf_g[:, None] * head_w
    wh[:E] = wh_f
    wh[E] = wh_f.sum(0)
    wh[E + 1] = lnf_b @ head_w + head_b

    cast = lambda x: np.ascontiguousarray(x, dtype=BF16)
    return dict(wqk=cast(wqk), wv=cast(wv), wpr=cast(wpr),
                wf1=cast(wf1), wf2=cast(wf2), wh=cast(wh))


def make_in_maps(inputs, n_layers=L, include_head=True):
    tokens = np.asarray(inputs["tokens"]).astype(np.int64)
    wte = np.asarray(inputs["wte"], np.float32)
    wpe = np.asarray(inputs["wpe"], np.float32)
    x0 = wte[tokens] + wpe[None, :, :]           # [B, T, E]
    x0 = x0.reshape(NCORES, TC, E)

    w = _prep_weights(inputs)

    mask = np.where(np.arange(128)[:, None] <= np.arange(128)[None, :],
                    np.float32(0.0), np.float32(NEG))

    LD = max(n_layers, 1)
    in_maps = []
    for c in range(NCORES):
        rem = np.full((128, 1), 0.0 if c % 2 == 1 else NEG, np.float32)
        m = {
            "x0T": np.ascontiguousarray(x0[c].T),
            "wqk": w["wqk"][:LD], "wv": w["wv"][:LD], "wpr": w["wpr"][:LD],
            "wf1": w["wf1"][:LD], "wf2": w["wf2"][:LD],
            "mask_tri": mask, "rem_bias": rem,
        }
        if include_head:
            m["wh"] = w["wh"]
        in_maps.append(m)
    return in_maps


def kernel(**inputs):
    key = "full"
    if key not in _CACHE:
        _CACHE[key] = build_nc(L, True)
    nc = _CACHE[key]
    in_maps = make_in_maps(inputs)
    res = run_bass_kernel_spmd(nc, in_maps, core_ids=list(range(NCORES)))
    outs = [res.results[c]["out"] for c in range(NCORES)]
    logits = np.stack(outs, 0).reshape(B, T, V).astype(np.float32)
    return logits


# revision 27
# speedup vs baseline: 12.0606x; 12.0606x over previous
"""Trainium2 Bass kernel for an 8-layer GPT-2-style transformer (NanoShakes).

Sharding: 8-way sequence-data-parallel. The 4x1024 tokens are split into 8
chunks of 512 (4 batch rows x 2 halves). Core 2r holds the first half of row
r, core 2r+1 the second half. Attention for second-half queries needs the
first half's K/V, exchanged with a pairwise AllGather each layer; first-half
cores mask out the (unneeded) remote block via an additive -1e9 exp bias, so
one uniform SPMD program runs on all cores.

On-device layout: activations are kept feature-major ("transposed", [E, T])
so every dense matmul uses the natural [E_in, E_out] weight as the stationary
operand and never needs an on-chip transpose. LayerNorm gains are folded into
the following weight matrix on the host; the LN mean subtraction and all
biases are applied through augmented contraction rows (extra k-rows of the
matmul). Matmul compute is bf16 with f32 accumulation; the residual stream
stays f32.
"""

import sys

sys.path.insert(0, "/opt/trn_rl_repo")

import contextlib

import numpy as np
import ml_dtypes

import concourse.bass as bass
import concourse.bacc as bacc
import concourse.tile as tile
from concourse import mybir
from concourse.bass_utils import run_bass_kernel_spmd

BF16 = ml_dtypes.bfloat16
F32 = mybir.dt.float32
BF = mybir.dt.bfloat16

V, T, E, L, H = 32000, 1024, 1024, 8, 16
DH = E // H
EPS = 1e-5
B = 4
NCORES = 8
TC = 512          # tokens per core
KT = 8            # k-tiles over E
NEG = -1.0e9

_CACHE = {}


# ----------------------------------------------------------------------------
# device program
# ----------------------------------------------------------------------------

def build_nc(n_layers=L, include_head=True, debug_tap=None, sim_local=False):
    nc = bacc.Bacc("TRN2", target_bir_lowering=False, debug=False,
                   num_devices=1 if sim_local else NCORES)

    LD = max(n_layers, 1)
    x0T_d = nc.dram_tensor("x0T", [E, TC], F32, kind="ExternalInput")
    wqk_d = nc.dram_tensor("wqk", [LD, E + 2, 2 * E], BF, kind="ExternalInput")
    wv_d = nc.dram_tensor("wv", [LD, E + 2, E], BF, kind="ExternalInput")
    wpr_d = nc.dram_tensor("wpr", [LD, E + 1, E], BF, kind="ExternalInput")
    wf1_d = nc.dram_tensor("wf1", [LD, E + 2, 4 * E], BF, kind="ExternalInput")
    wf2_d = nc.dram_tensor("wf2", [LD, 4 * E + 1, E], BF, kind="ExternalInput")
    wh_d = (nc.dram_tensor("wh", [E + 2, V], BF, kind="ExternalInput")
            if include_head else None)
    mask_d = nc.dram_tensor("mask_tri", [128, 128], F32, kind="ExternalInput")
    rb_d = nc.dram_tensor("rem_bias", [128, 1], F32, kind="ExternalInput")
    if include_head:
        out_d = nc.dram_tensor("out", [TC, V], F32, kind="ExternalOutput")
    elif debug_tap is not None:
        out_d = nc.dram_tensor("out", [4096, TC], F32, kind="ExternalOutput")
    else:
        out_d = nc.dram_tensor("out", [E, TC], F32, kind="ExternalOutput")

    with tile.TileContext(nc) as tc:
        _body(nc, tc, n_layers, include_head,
              x0T_d, wqk_d, wv_d, wpr_d, wf1_d, wf2_d, wh_d, mask_d, rb_d,
              out_d, debug_tap, sim_local)
    nc.compile()
    return nc


def _body(nc, tc, n_layers, include_head,
          x0T_d, wqk_d, wv_d, wpr_d, wf1_d, wf2_d, wh_d, mask_d, rb_d, out_d,
          debug_tap=None, sim_local=False):
    AG = mybir.AluOpType
    AF = mybir.ActivationFunctionType
    ts = bass.ts

    ctx = contextlib.ExitStack()
    with ctx:
        persist = ctx.enter_context(tc.tile_pool(name="persist", bufs=1))
        act = ctx.enter_context(tc.tile_pool(name="act", bufs=1))
        wstr = ctx.enter_context(tc.tile_pool(name="wstr", bufs=2))
        wf2p = ctx.enter_context(tc.tile_pool(name="wf2p", bufs=4))
        waug = ctx.enter_context(tc.tile_pool(name="waug", bufs=2))
        ppool = ctx.enter_context(tc.tile_pool(name="ppool", bufs=3))
        small = ctx.enter_context(tc.tile_pool(name="small", bufs=2))
        rbp = ctx.enter_context(tc.tile_pool(name="rbp", bufs=2))
        lop = ctx.enter_context(tc.tile_pool(name="lop", bufs=3))
        dram = ctx.enter_context(tc.tile_pool(name="dram", bufs=2,
                                              space="DRAM"))
        psp = ctx.enter_context(tc.tile_pool(name="psp", bufs=8,
                                             space="PSUM"))

        def ptile():
            return psp.tile([128, TC], F32, tag="ps", name="ps")

        # ---- persistent tiles -------------------------------------------
        xT = persist.tile([128, KT, TC], F32)         # residual stream
        ones_col_f = persist.tile([128, 1], F32)      # lhsT for col-sums
        ones_col_b = persist.tile([128, 1], BF)
        ones_row_f = persist.tile([1, 128], F32)      # lhsT for broadcasts
        ones_row_b = persist.tile([1, TC], BF)        # rhs aug row (biases)
        mask_tri = persist.tile([128, 128], F32)
        rem_bias = persist.tile([128, 1], F32)
        zero_bias = persist.tile([128, 1], F32)
        eps_tile = persist.tile([1, 1], F32)
        nc.vector.memset(eps_tile[:], EPS)

        nc.vector.memset(ones_col_f[:], 1.0)
        nc.vector.memset(ones_col_b[:], 1.0)
        nc.vector.memset(ones_row_f[:], 1.0)
        nc.vector.memset(ones_row_b[:], 1.0)
        nc.vector.memset(zero_bias[:], 0.0)
        nc.sync.dma_start(mask_tri[:], mask_d[:])
        nc.sync.dma_start(rem_bias[:], rb_d[:])

        # load embedding (transposed):  [E, TC] -> [128, KT, TC]
        nc.sync.dma_start(
            xT[:], x0T_d.ap().rearrange("(kt p) t -> p kt t", p=128))

        # ------------------------------------------------------------------
        def layernorm():
            """LN over the partition (feature) axis of xT.

            Returns (xa, augx): xa = (x * rstd) in bf16, and augx [2, TC]
            rhs-rows [c; ones] with c = -mu * rstd; the following matmul's
            host-prepared weights carry the matching column-sum and bias
            rows that complete the normalization.
            """
            s1 = ptile()
            s2 = ptile()
            for k in range(KT):
                sq = ppool.tile([128, TC], BF, tag="sq")
                nc.scalar.activation(sq[:], xT[:, k, :], AF.Square)
                nc.tensor.matmul(s1[0:1, :], ones_col_f[:], xT[:, k, :],
                                 start=(k == 0), stop=(k == KT - 1))
                nc.tensor.matmul(s2[0:1, :], ones_col_b[:], sq[:],
                                 start=(k == 0), stop=(k == KT - 1))
            mu = small.tile([1, TC], F32, tag="mu")
            var = small.tile([1, TC], F32, tag="var")
            a = small.tile([1, TC], F32, tag="a")
            tmp = small.tile([1, TC], F32, tag="tmp")
            nc.vector.tensor_scalar_mul(mu[:], s1[0:1, :], 1.0 / E)
            nc.vector.tensor_scalar_mul(var[:], s2[0:1, :], 1.0 / E)
            nc.vector.tensor_tensor(tmp[:], mu[:], mu[:], AG.mult)
            nc.vector.tensor_tensor(var[:], var[:], tmp[:], AG.subtract)
            # rstd = exp(-0.5 * ln(var + eps))  (avoids sqrt table swap)
            nc.scalar.activation(tmp[:], var[:], AF.Ln, bias=eps_tile[:])
            nc.scalar.activation(a[:], tmp[:], AF.Exp, scale=-0.5)
            augx = small.tile([2, TC], BF, tag="augx")
            nc.vector.tensor_tensor(tmp[:], mu[:], a[:], AG.mult)
            nc.vector.tensor_scalar_mul(augx[0:1, :], tmp[:], -1.0)
            nc.sync.dma_start(augx[1:2, :], ones_row_b[0:1, :])
            # broadcast a over partitions (k=1 matmul), then apply
            ab = ptile()
            nc.tensor.matmul(ab[:], ones_row_f[:], a[:], start=True, stop=True)
            xa = act.tile([128, KT, TC], BF, tag="xa")
            for k in range(KT):
                nc.vector.tensor_tensor(xa[:, k, :], xT[:, k, :], ab[:],
                                        AG.mult)
            return xa, augx

        # ------------------------------------------------------------------
        def dense_wT(xa, augx, w_dram, waug_dram, out_cb, n_cols, aug_k):
            """out_T = W'.T @ xhat (+ aug rows), W' streamed in 1024-col
            chunks. out_cb(m_global, psum_tile) consumes each 128-row
            output tile."""
            n_chunks = n_cols // 1024
            for ch in range(n_chunks):
                cs = slice(1024 * ch, 1024 * (ch + 1))
                w_t = wstr.tile([128, KT, 1024], BF, tag="wstream")
                nc.sync.dma_start(
                    w_t[:],
                    w_dram[0:E, cs].rearrange("(kt p) n -> p kt n", p=128))
                w_a = waug.tile([aug_k, 1024], BF, tag="waug")
                nc.sync.dma_start(w_a[:], waug_dram[:, cs])
                for mm in range(8):
                    ps = ptile()
                    for k in range(KT):
                        nc.tensor.matmul(ps[:], w_t[:, k, ts(mm, 128)],
                                         xa[:, k, :],
                                         start=(k == 0), stop=False)
                    nc.tensor.matmul(
                        ps[:], w_a[:, ts(mm, 128)],
                        augx[:] if aug_k == 2 else ones_row_b[0:1, :],
                        start=False, stop=True)
                    out_cb(8 * ch + mm, ps)

        def dbg_rows(r0, nrows):
            return out_d[r0:r0 + nrows, :]

        # ------------------------------------------------------------------
        for l in range(n_layers):
            # ================= LN1 + QKV =================================
            xa, augx = layernorm()
            if debug_tap == "xa" and l == 0:
                nc.gpsimd.dma_start(
                    dbg_rows(0, E).rearrange("(kt p) t -> p kt t", p=128),
                    xa[:])
                nc.gpsimd.dma_start(dbg_rows(3072, 2), augx[:])

            qk = act.tile([128, 16, TC], BF, tag="qk")

            def qk_out(m, ps):
                nc.vector.tensor_copy(qk[:, m, :], ps[:])

            dense_wT(xa, augx, wqk_d[l], wqk_d[l, E:E + 2, :], qk_out,
                     2 * E, 2)
            if debug_tap == "qk" and l == 0:
                nc.gpsimd.dma_start(
                    dbg_rows(0, 2 * E).rearrange("(kt p) t -> p kt t", p=128),
                    qk[:])

            # V in natural layout [tokens, E] with an extra ones column per
            # head (so the PV matmul also emits the softmax denominator).
            vloc = act.tile([128, 4, H, DH + 1], BF, tag="vloc")
            for tt in range(4):
                nc.vector.memset(vloc[:, tt, :, DH:DH + 1], 1.0)
            for nv in range(2):
                wv_t = wstr.tile([128, KT, 512], BF, tag="wstream")
                nc.sync.dma_start(
                    wv_t[:],
                    wv_d[l, 0:E, ts(nv, 512)].rearrange(
                        "(kt p) n -> p kt n", p=128))
                wv_aug = waug.tile([2, 512], BF, tag="waug")
                nc.sync.dma_start(wv_aug[:], wv_d[l, E:E + 2, ts(nv, 512)])
                for tt in range(4):
                    ps = ptile()
                    for k in range(KT):
                        nc.tensor.matmul(ps[:], xa[:, k, ts(tt, 128)],
                                         wv_t[:, k, :],
                                         start=(k == 0), stop=False)
                    nc.tensor.matmul(ps[:], augx[:, ts(tt, 128)], wv_aug[:],
                                     start=False, stop=True)
                    nc.vector.tensor_copy(
                        vloc[:, tt, ts(nv, 8), 0:DH],
                        ps[:].rearrange("p (h d) -> p h d", d=DH))

            if debug_tap == "v" and l == 0:
                for i in range(4):
                    nc.gpsimd.dma_start(
                        dbg_rows(256 * i, 256).rearrange(
                            "(p two) c -> p (two c)", two=2),
                        vloc[:, i, :, 0:DH])

            # ================= K/V exchange ==============================
            kv_send = dram.tile([2048, TC], BF, tag="kv_send")
            for j in range(8):
                nc.sync.dma_start(kv_send[128 * j:128 * (j + 1), :],
                                  qk[:, 8 + j, :])
            for i in range(4):
                nc.sync.dma_start(
                    kv_send[1024 + 256 * i:1024 + 256 * (i + 1), :]
                    .rearrange("(p two) c -> p (two c)", two=2),
                    vloc[:, i, :, 0:DH])
            kv_all = dram.tile([2, 2048, TC], BF, tag="kv_all")
            if sim_local:
                nc.gpsimd.dma_start(kv_all[0], kv_send[:])
            else:
                nc.gpsimd.collective_compute(
                    "AllGather", AG.bypass,
                    replica_groups=[[0, 1], [2, 3], [4, 5], [6, 7]],
                    ins=[kv_send[:].opt()],
                    outs=[kv_all[:].opt()],
                )
            krem = act.tile([128, 8, TC], BF, tag="krem")
            for j in range(8):
                nc.sync.dma_start(krem[:, j, :],
                                  kv_all[0, 128 * j:128 * (j + 1), :])
            vrem = act.tile([128, 4, H, DH + 1], BF, tag="vrem")
            for i in range(4):
                nc.vector.memset(vrem[:, i, :, DH:DH + 1], 1.0)
                nc.sync.dma_start(
                    vrem[:, i, :, 0:DH],
                    kv_all[0, 1024 + 256 * i:1024 + 256 * (i + 1), :]
                    .rearrange("(p two) c -> p (two c)", two=2))

            # ================= attention =================================
            oT = act.tile([128, KT, TC], BF, tag="oT")
            for h in range(H):
                pb = (h % 2) * 64
                q_h = qk[pb:pb + 64, h // 2, :]
                o_aug = ptile()
                # remote k-tiles (keys 0..511 of the pair's first half)
                for i in range(4):
                    st = ptile()
                    nc.tensor.matmul(st[:], krem[pb:pb + 64, h // 2,
                                                 ts(i, 128)],
                                     q_h, start=True, stop=True)
                    p = ppool.tile([128, TC], BF, tag="p")
                    nc.scalar.activation(p[:], st[:], AF.Exp,
                                         bias=rem_bias[:])
                    nc.tensor.matmul(o_aug[0:65, :], vrem[:, i, h, :], p[:],
                                     start=(i == 0), stop=False)
                # local k-tiles (causal)
                for j in range(4):
                    n = TC - 128 * j
                    st = ptile()
                    nc.tensor.matmul(st[:, 0:n],
                                     qk[pb:pb + 64, 8 + h // 2, ts(j, 128)],
                                     q_h[:, 128 * j:TC],
                                     start=True, stop=True)
                    nc.vector.tensor_tensor(st[:, 0:128], st[:, 0:128],
                                            mask_tri[:], AG.add)
                    p = ppool.tile([128, TC], BF, tag="p")
                    nc.scalar.activation(p[:, 0:n], st[:, 0:n], AF.Exp,
                                         bias=zero_bias[:])
                    nc.tensor.matmul(o_aug[0:65, 128 * j:TC],
                                     vloc[:, j, h, :], p[:, 0:n],
                                     start=False, stop=(j == 3))
                # normalize: o / den  (den = row 64 of o_aug)
                den_sb = rbp.tile([1, TC], F32, tag="den_sb")
                nc.vector.tensor_copy(den_sb[:], o_aug[64:65, :])
                rden = rbp.tile([1, TC], F32, tag="rden")
                nc.vector.reciprocal_approx_fast(rden[:], den_sb[:])
                rbps = ptile()
                nc.tensor.matmul(rbps[0:64, :], ones_row_f[0:1, 0:64],
                                 rden[:], start=True, stop=True)
                rb = rbp.tile([64, TC], F32, tag="rb")
                nc.scalar.copy(rb[:], rbps[0:64, :])
                nc.vector.tensor_tensor(oT[pb:pb + 64, h // 2, :],
                                        o_aug[0:64, :], rb[:], AG.mult)
                if debug_tap == "att0" and l == 0 and h == 0:
                    dbg = persist.tile([128, 3, TC], F32, name="dbg")
                    nc.vector.tensor_copy(dbg[0:65, 0, :], o_aug[0:65, :])
                    nc.vector.tensor_copy(dbg[0:1, 1, :], rden[:])
                    nc.vector.tensor_copy(dbg[0:64, 2, :], rb[:])
                    nc.gpsimd.dma_start(
                        dbg_rows(0, 384).rearrange("(k p) t -> p k t", p=128),
                        dbg[:])

            if debug_tap == "o" and l == 0:
                nc.gpsimd.dma_start(
                    dbg_rows(0, E).rearrange("(kt p) t -> p kt t", p=128),
                    oT[:])

            # ================= proj + residual ===========================
            def proj_out(m, ps):
                nc.vector.tensor_tensor(xT[:, m, :], xT[:, m, :], ps[:],
                                        AG.add)

            dense_wT(oT, None, wpr_d[l], wpr_d[l, E:E + 1, :], proj_out,
                     E, 1)

            # ================= LN2 + FFN =================================
            xa2, augx2 = layernorm()
            hT0 = act.tile([128, 16, TC], BF, tag="hT0")
            hT1 = act.tile([128, 16, TC], BF, tag="hT1")

            def ff1_out(m, ps):
                dst = hT0 if m < 16 else hT1
                nc.scalar.activation(dst[:, m % 16, :], ps[:], AF.Relu)

            dense_wT(xa2, augx2, wf1_d[l], wf1_d[l, E:E + 2, :], ff1_out,
                     4 * E, 2)

            ffps = [ptile() for _ in range(KT)]
            for kg in range(32):
                wf2 = wf2p.tile([128, E], BF, tag="wf2")
                nc.sync.dma_start(wf2[:],
                                  wf2_d[l, 128 * kg:128 * (kg + 1), :])
                src = hT0 if kg < 16 else hT1
                for m in range(KT):
                    nc.tensor.matmul(ffps[m][:], wf2[:, ts(m, 128)],
                                     src[:, kg % 16, :],
                                     start=(kg == 0), stop=False)
            wf2_aug = waug.tile([1, E], BF, tag="waug")
            nc.sync.dma_start(wf2_aug[:], wf2_d[l, 4 * E:4 * E + 1, :])
            for m in range(KT):
                nc.tensor.matmul(ffps[m][:], wf2_aug[:, ts(m, 128)],
                                 ones_row_b[0:1, :], start=False, stop=True)
                nc.vector.tensor_tensor(xT[:, m, :], xT[:, m, :],
                                        ffps[m][:], AG.add)

        # ================= final LN + head ===============================
        if debug_tap is not None:
            return
        if not include_head:
            for k in range(KT):
                nc.sync.dma_start(
                    out_d.ap().rearrange("(kt p) t -> p kt t", p=128)[:, k, :],
                    xT[:, k, :])
            return

        xaf, augxf = layernorm()
        vchunks = [(i * 512, 512) for i in range(62)] + [(62 * 512, 256)]
        for ci, (v0, vn) in enumerate(vchunks):
            wh_t = wstr.tile([128, KT, 512], BF, tag="wstream")
            nc.sync.dma_start(
                wh_t[:, :, 0:vn],
                wh_d[0:E, v0:v0 + vn].rearrange("(kt p) n -> p kt n", p=128))
            wh_aug = waug.tile([2, 512], BF, tag="waug")
            nc.sync.dma_start(wh_aug[:, 0:vn], wh_d[E:E + 2, v0:v0 + vn])
            for tt in range(4):
                ps = ptile()
                for k in range(KT):
                    nc.tensor.matmul(ps[:, 0:vn], xaf[:, k, ts(tt, 128)],
                                     wh_t[:, k, 0:vn],
                                     start=(k == 0), stop=False)
                nc.tensor.matmul(ps[:, 0:vn], augxf[:, ts(tt, 128)],
                                 wh_aug[:, 0:vn], start=False, stop=True)
                lo = lop.tile([128, 512], F32, tag="lo")
                if tt % 2 == 0:
                    nc.vector.tensor_copy(lo[:, 0:vn], ps[:, 0:vn])
                else:
                    nc.scalar.copy(lo[:, 0:vn], ps[:, 0:vn])
                nc.sync.dma_start(out_d[ts(tt, 128), v0:v0 + vn],
                                  lo[:, 0:vn])


# ----------------------------------------------------------------------------
# host side
# ----------------------------------------------------------------------------

def _prep_weights(inputs):
    """Fold LN gains/biases into weights, build augmented rows, cast bf16."""
    f = lambda k: np.asarray(inputs[k], np.float32)
    qkv_w, proj_w, proj_b = f("qkv_w"), f("proj_w"), f("proj_b")
    ff1_w, ff1_b, ff2_w, ff2_b = f("ff1_w"), f("ff1_b"), f("ff2_w"), f("ff2_b")
    ln1_g, ln1_b = f("ln1_g"), f("ln1_b")
    ln2_g, ln2_b = f("ln2_g"), f("ln2_b")
    lnf_g, lnf_b = f("lnf_g"), f("lnf_b")
    head_w, head_b = f("head_w"), f("head_b")

    qscale = DH ** -0.5

    wqk = np.empty((L, E + 2, 2 * E), np.float32)
    wv = np.empty((L, E + 2, E), np.float32)
    wpr = np.empty((L, E + 1, E), np.float32)
    wf1 = np.empty((L, E + 2, 4 * E), np.float32)
    wf2 = np.empty((L, 4 * E + 1, E), np.float32)
    for l in range(L):
        wqk_s = qkv_w[l][:, :2 * E].copy()
        wqk_s[:, :E] *= qscale
        wqk_f = ln1_g[l][:, None] * wqk_s
        wqk[l, :E] = wqk_f
        wqk[l, E] = wqk_f.sum(0)
        wqk[l, E + 1] = ln1_b[l] @ wqk_s

        wv_s = qkv_w[l][:, 2 * E:]
        wv_f = ln1_g[l][:, None] * wv_s
        wv[l, :E] = wv_f
        wv[l, E] = wv_f.sum(0)
        wv[l, E + 1] = ln1_b[l] @ wv_s

        wpr[l, :E] = proj_w[l]
        wpr[l, E] = proj_b[l]

        wf1_f = ln2_g[l][:, None] * ff1_w[l]
        wf1[l, :E] = wf1_f
        wf1[l, E] = wf1_f.sum(0)
        wf1[l, E + 1] = ln2_b[l] @ ff1_w[l] + ff1_b[l]

        wf2[l, :4 * E] = ff2_w[l]
        wf2[l, 4 * E] = ff2_b[l]

    wh = np.empty((E + 2, V), np.float32)
    wh_f = lnf_g[:, None] * head_w
    wh[:E] = wh_f
    wh[E] = wh_f.sum(0)
    wh[E + 1] = lnf_b @ head_w + head_b

    cast = lambda x: np.ascontiguousarray(x, dtype=BF16)
    return dict(wqk=cast(wqk), wv=cast(wv), wpr=cast(wpr),
                wf1=cast(wf1), wf2=cast(wf2), wh=cast(wh))


def make_in_maps(inputs, n_layers=L, include_head=True):
    tokens = np.asarray(inputs["tokens"]).astype(np.int64)
    wte = np.asarray(inputs["wte"], np.float32)
    wpe = np.asarray(inputs["wpe"], np.float32)
    x0 = wte[tokens] + wpe[None, :, :]           # [B, T, E]
    x0 = x0.reshape(NCORES, TC, E)

    w = _prep_weights(inputs)

    mask = np.where(np.arange(128)[:, None] <= np.arange(128)[None, :],
                    np.float32(0.0), np.float32(NEG))

    LD = max(n_layers, 1)
    in_maps = []
    for c in range(NCORES):
        rem = np.full((128, 1), 0.0 if c % 2 == 1 else NEG, np.float32)
        m = {
            "x0T": np.ascontiguousarray(x0[c].T),
            "wqk": w["wqk"][:LD], "wv": w["wv"][:LD], "wpr": w["wpr"][:LD],
            "wf1": w["wf1"][:LD], "wf2": w["wf2"][:LD],
            "mask_tri": mask, "rem_bias": rem,
        }
        if include_head:
            m["wh"] = w["wh"]
        in_maps.append(m)
    return in_maps


def kernel(**inputs):
    key = "full"
    if key not in _CACHE:
        _CACHE[key] = build_nc(L, True)
    nc = _CACHE[key]
    in_maps = make_in_maps(inputs)
    res = run_bass_kernel_spmd(nc, in_maps, core_ids=list(range(NCORES)))
    outs = [res.results[c]["out"] for c in range(NCORES)]
    logits = np.stack(outs, 0).reshape(B, T, V).astype(np.float32)
    return logits
